# revision 32
# baseline (speedup 1.0000x reference)
"""GroupedQueryAttention Bass/Tile kernel for 8 TRN2 NeuronCores.

Sharding: data-parallel over (batch, query-quarter): core i -> batch i//4,
query rows (i%4)*512 .. +512. Each core computes all 16 heads for its 512
query rows fully on-device: QKV projections, QK-LayerNorm, attention in a
transposed score layout (scoresT[kv, q], so attn@V needs no transposes),
softmax denominators via ones-column matmuls col-tiled next to the AV
matmuls, then the out-projection. The 16 heads are processed as 8
row-tiled pairs on the 128x128 PE array (head_dim=64 -> two heads share
the contraction rows). Host work is layout staging only (transpose / bf16
cast / head permutation) plus the final shard concat.
"""
import sys

if "/opt/trn_rl_repo" not in sys.path:
    sys.path.insert(0, "/opt/trn_rl_repo")

import numpy as np
import ml_dtypes

BF16NP = ml_dtypes.bfloat16
DIM = 1024
NH = 16
NG = 4
HD = 64
HPG = 4
B = 2
SEQ = 2048
QSH = 512          # query rows per core
NDEV = 8
NPAIR = 8          # head pairs
NKC = 16           # kv chunks of 128
LN_EPS = 1e-5
SCALE = 1.0 / 8.0  # 1/sqrt(HD)


def _make_perm():
    # head pair p holds orig heads (g_lo, h) and (g_hi, h) in its low/high
    # 64 partitions, where g_lo = 2*(p//4), g_hi = g_lo+1, h = p%4.
    p_ = np.zeros(DIM, np.int64)
    for pr in range(NPAIR):
        for half in range(2):
            g = 2 * (pr // 4) + half
            h = pr % 4
            src = g * (HPG * HD) + h * HD
            dst = pr * 128 + half * 64
            p_[dst:dst + 64] = np.arange(src, src + 64)
    return p_


PERM = _make_perm()

_cache = {}


def _build(fq_aff, fk_aff, b_q, b_k, b_v, b_o):
    """Build + compile the single-core Bass graph (shared by all 8 cores)."""
    import concourse.bass as bass
    import concourse.tile as tile
    from concourse import bacc, mybir

    dt = mybir.dt
    FP, BF = dt.float32, dt.bfloat16
    AF = mybir.ActivationFunctionType
    OP = mybir.AluOpType

    nc = bacc.Bacc("TRN2", target_bir_lowering=False, debug=False)

    def din(name, shape, dtype=BF):
        return nc.dram_tensor(name, shape, dtype, kind="ExternalInput").ap()

    xq = din("xqT", [DIM, QSH])
    xk = din("xkT", [DIM, SEQ])
    xv = din("xvT", [DIM, SEQ])
    wq = din("wqT", [DIM, DIM])
    wk = din("wkT", [DIM, 256])
    wv = din("wvT", [DIM, 256])
    wo = din("woT", [DIM, DIM])
    idn = din("ident", [128, 128])
    gq = din("gq", [128, DIM]) if fq_aff else None
    zq = din("zq", [128, DIM]) if fq_aff else None
    gk = din("gk", [128, 256]) if fk_aff else None
    zk = din("zk", [128, 256]) if fk_aff else None
    rbq = din("rbq", [1, DIM]) if b_q else None
    rbk = din("rbk", [1, 256]) if b_k else None
    rbv = din("rbv", [1, 256]) if b_v else None
    rbo = din("rbo", [1, DIM]) if b_o else None
    out_d = nc.dram_tensor("out", [QSH, DIM], FP, kind="ExternalOutput").ap()

    with tile.TileContext(nc) as tc:
        from contextlib import ExitStack
        with ExitStack() as ctx:
            pers = ctx.enter_context(tc.tile_pool(name="pers", bufs=1))
            wexp = ctx.enter_context(tc.tile_pool(name="wexp", bufs=6))
            wrec = ctx.enter_context(tc.tile_pool(name="wrec", bufs=2))

            # ---- stage inputs into SBUF ----
            xq_s = pers.tile([128, 8 * QSH], BF, tag="xq")
            xk_s = pers.tile([128, 8 * SEQ], BF, tag="xk")
            xv_s = pers.tile([128, 8 * SEQ], BF, tag="xv")
            wq_s = pers.tile([128, 8 * DIM], BF, tag="wq")
            wk_s = pers.tile([128, 8 * 256], BF, tag="wk")
            wv_s = pers.tile([128, 8 * 256], BF, tag="wv")
            wo_s = pers.tile([128, 8 * DIM], BF, tag="wo")
            id_s = pers.tile([128, 128], BF, tag="ident")
            nc.sync.dma_start(id_s[:], idn[:])
            # q-projection inputs first so the PE can start ~10us in, with
            # the k/v/wo streams landing under the early compute.
            for mc in range(8):
                r = slice(mc * 128, (mc + 1) * 128)
                nc.sync.dma_start(wq_s[:, mc * DIM:(mc + 1) * DIM], wq[r, :])
                nc.sync.dma_start(xq_s[:, mc * QSH:(mc + 1) * QSH], xq[r, :])
            for mc in range(8):
                r = slice(mc * 128, (mc + 1) * 128)
                nc.sync.dma_start(wk_s[:, mc * 256:(mc + 1) * 256], wk[r, :])
                nc.sync.dma_start(wv_s[:, mc * 256:(mc + 1) * 256], wv[r, :])
                nc.sync.dma_start(xk_s[:, mc * SEQ:(mc + 1) * SEQ], xk[r, :])
            for mc in range(8):
                r = slice(mc * 128, (mc + 1) * 128)
                nc.sync.dma_start(xv_s[:, mc * SEQ:(mc + 1) * SEQ], xv[r, :])
            for mc in range(8):
                r = slice(mc * 128, (mc + 1) * 128)
                nc.sync.dma_start(wo_s[:, mc * DIM:(mc + 1) * DIM], wo[r, :])
            gq_s = zq_s = gk_s = zk_s = None
            if fq_aff:
                gq_s = pers.tile([128, DIM], BF, tag="gq")
                zq_s = pers.tile([128, DIM], BF, tag="zq")
                nc.sync.dma_start(gq_s[:], gq[:])
                nc.sync.dma_start(zq_s[:], zq[:])
            if fk_aff:
                gk_s = pers.tile([128, 256], BF, tag="gk")
                zk_s = pers.tile([128, 256], BF, tag="zk")
                nc.sync.dma_start(gk_s[:], gk[:])
                nc.sync.dma_start(zk_s[:], zk[:])
            rbq_s = rbk_s = rbv_s = rbo_s = ones_b = None
            if b_q or b_k or b_v or b_o:
                ones_b = pers.tile([1, 128], BF, tag="ones_b")
                nc.vector.memset(ones_b[:], 1.0)
            if b_q:
                rbq_s = pers.tile([1, DIM], BF, tag="rbq")
                nc.sync.dma_start(rbq_s[:], rbq[:])
            if b_k:
                rbk_s = pers.tile([1, 256], BF, tag="rbk")
                nc.sync.dma_start(rbk_s[:], rbk[:])
            if b_v:
                rbv_s = pers.tile([1, 256], BF, tag="rbv")
                nc.sync.dma_start(rbv_s[:], rbv[:])
            if b_o:
                rbo_s = pers.tile([1, DIM], BF, tag="rbo")
                nc.sync.dma_start(rbo_s[:], rbo[:])

            ones_c = pers.tile([128, 1], BF, tag="ones_c")    # den lhsT
            nc.vector.memset(ones_c[:], 1.0)
            ones_r = pers.tile([128, 64], BF, tag="ones_r")   # bcast lhsT
            nc.vector.memset(ones_r[:], 1.0)

            q_nat = pers.tile([128, 4 * DIM], BF, tag="q_nat")
            k_nat = pers.tile([128, 16 * 256], BF, tag="k_nat")
            v_all = pers.tile([128, 16 * 256], BF, tag="v_all")
            qT = pers.tile([128, NPAIR * QSH], BF, tag="qT")
            kT = pers.tile([128, 2 * SEQ], BF, tag="kT")
            oT = pers.tile([128, NPAIR * QSH], BF, tag="oT")

            # ============ phase 1: projections + LN + transposes ============
            with tc.tile_pool(name="pp1", bufs=2, space="PSUM") as pp1, \
                 tc.tile_pool(name="ppt", bufs=4, space="PSUM") as ppt:

                # q projection -> q_nat[q(128 x 4qc), qc*1024 + o']
                for qc in range(4):
                    for nci in range(2):
                        ps = pp1.tile([128, 512], FP, tag="pj")
                        for mc in range(8):
                            nc.tensor.matmul(
                                ps[:],
                                lhsT=xq_s[:, mc * QSH + qc * 128: mc * QSH + (qc + 1) * 128],
                                rhs=wq_s[:, mc * DIM + nci * 512: mc * DIM + (nci + 1) * 512],
                                start=(mc == 0), stop=(mc == 7 and not b_q))
                        if b_q:
                            nc.tensor.matmul(ps[:], lhsT=ones_b[:],
                                             rhs=rbq_s[:, nci * 512:(nci + 1) * 512],
                                             start=False, stop=True)
                        nc.scalar.copy(q_nat[:, qc * DIM + nci * 512: qc * DIM + (nci + 1) * 512], ps[:])

                # k projection -> k_nat[kv(128 x 16kc), kc*256 + gd]
                # v projection -> v_all (same layout)
                for (x_s, w_s, rb_s, dst, nz) in ((xk_s, wk_s, rbk_s, k_nat, b_k),
                                                  (xv_s, wv_s, rbv_s, v_all, b_v)):
                    for ic in range(8):
                        ps = pp1.tile([128, 512], FP, tag="pj")
                        for half in range(2):
                            c = 2 * ic + half
                            o = slice(half * 256, (half + 1) * 256)
                            for mc in range(8):
                                nc.tensor.matmul(
                                    ps[:, o],
                                    lhsT=x_s[:, mc * SEQ + c * 128: mc * SEQ + (c + 1) * 128],
                                    rhs=w_s[:, mc * 256:(mc + 1) * 256],
                                    start=(mc == 0), stop=(mc == 7 and not nz))
                            if nz:
                                nc.tensor.matmul(ps[:, o], lhsT=ones_b[:], rhs=rb_s[:],
                                                 start=False, stop=True)
                        nc.scalar.copy(dst[:, ic * 512:(ic + 1) * 512], ps[:])

                # ---- LayerNorm over head_dim (free-dim segments of 64) ----
                # Stats via DVE reduces; the per-segment (scale, shift) pair is
                # broadcast to full [128, 4096] tiles on the otherwise-idle
                # GPSIMD engine (step-0 inner AP), then applied with two big
                # 2x-mode tensor_tensor ops. scrA/scrB reuse dead tiles.
                NSEG = 64

                def bcast64(ap):
                    flat = ap.rearrange("p a b -> p (a b)")
                    return bass.AP(flat.tensor, flat.offset, flat.ap.copy() + [[0, HD]])

                sqb = pers.tile([128, 4096], BF, tag="sqb")
                for (big, tg, g_s, z_s, aff, ngrp) in (
                        (q_nat, "q", gq_s, zq_s, fq_aff, 4),
                        (k_nat, "k", gk_s, zk_s, fk_aff, 16)):
                    mean = pers.tile([128, NSEG, 1], FP, tag="mn_" + tg)
                    veps = pers.tile([128, NSEG, 1], FP, tag="ve_" + tg)
                    aa = pers.tile([128, NSEG, 1], FP, tag="aa_" + tg)
                    bb = pers.tile([128, NSEG, 1], FP, tag="bb_" + tg)
                    nc.vector.tensor_tensor(sqb[:], big[:], big[:], op=OP.mult)
                    nc.vector.reduce_sum(
                        mean[:], big[:].rearrange("p (g d) -> p g d", d=HD),
                        axis=mybir.AxisListType.X)
                    nc.vector.reduce_sum(
                        veps[:], sqb[:].rearrange("p (g d) -> p g d", d=HD),
                        axis=mybir.AxisListType.X)
                    nc.vector.tensor_scalar(mean[:], mean[:], 1.0 / HD, None, OP.mult)
                    nc.vector.tensor_scalar(veps[:], veps[:], 1.0 / HD, LN_EPS,
                                            OP.mult, OP.add)
                    nc.vector.tensor_tensor(bb[:], mean[:], mean[:], op=OP.mult)
                    nc.vector.tensor_tensor(veps[:], veps[:], bb[:], op=OP.subtract)
                    nc.scalar.activation(veps[:], veps[:], AF.Sqrt)
                    nc.vector.reciprocal(aa[:], veps[:])
                    nc.vector.tensor_tensor(bb[:], mean[:], aa[:], op=OP.mult)
                    nc.vector.tensor_scalar(bb[:], bb[:], -1.0, None, OP.mult)
                    big3 = big[:].rearrange("p (g d) -> p g d", d=HD)
                    nc.vector.tensor_tensor(big3, big3, bcast64(aa[:]), op=OP.mult)
                    nc.vector.tensor_tensor(big3, big3, bcast64(bb[:]), op=OP.add)
                    if aff:
                        w = 4096 // ngrp
                        for s in range(ngrp):
                            sl = slice(s * w, (s + 1) * w)
                            nc.vector.tensor_tensor(big[:, sl], big[:, sl], g_s[:, 0:w], op=OP.mult)
                            nc.vector.tensor_tensor(big[:, sl], big[:, sl], z_s[:, 0:w], op=OP.add)

                # ---- PE transposes: q_nat -> qT, k_nat -> kT ----
                for pr in range(8):
                    for qc in range(4):
                        pt = ppt.tile([128, 128], BF, tag="tr")
                        nc.tensor.transpose(pt[:], q_nat[:, qc * DIM + pr * 128: qc * DIM + (pr + 1) * 128], id_s[:])
                        nc.any.tensor_copy(qT[:, pr * QSH + qc * 128: pr * QSH + (qc + 1) * 128], pt[:])
                for t in range(2):
                    for kc in range(16):
                        pt = ppt.tile([128, 128], BF, tag="tr")
                        nc.tensor.transpose(pt[:], k_nat[:, kc * 256 + t * 128: kc * 256 + (t + 1) * 128], id_s[:])
                        nc.any.tensor_copy(kT[:, t * SEQ + kc * 128: t * SEQ + (kc + 1) * 128], pt[:])

            # ============ phase 2: attention ============
            with tc.tile_pool(name="pps", bufs=2, space="PSUM") as pps, \
                 tc.tile_pool(name="ppav", bufs=2, space="PSUM") as ppav, \
                 tc.tile_pool(name="ppd", bufs=2, space="PSUM") as ppd:
                def finalize(pr, av, den):
                    # bf16-cast the two denominator rows, broadcast them over
                    # the pair's 64-partition halves with cheap bf16 matmuls,
                    # then one full-width reciprocal (cost is FD-bound) + mult.
                    dcb = wrec.tile([128, 512], BF, tag="dcb")
                    nc.vector.tensor_copy(dcb[0:1, :], den[0:1, :])
                    nc.vector.tensor_copy(dcb[32:33, :], den[32:33, :])
                    bc = ppd.tile([128, 512], FP, tag="dn")
                    nc.tensor.matmul(bc[0:64, :], lhsT=ones_r[0:1, :], rhs=dcb[0:1, :])
                    nc.tensor.matmul(bc[64:128, :], lhsT=ones_r[32:33, :], rhs=dcb[32:33, :])
                    rcb = wrec.tile([128, 512], FP, tag="rc")
                    nc.vector.reciprocal(rcb[:], bc[:])
                    nc.vector.tensor_tensor(oT[:, pr * QSH:(pr + 1) * QSH], av[:], rcb[:], op=OP.mult)

                fin_pend = None
                for pr in range(NPAIR):
                    t = pr // 4
                    g_lo = 2 * (pr // 4)
                    g_hi = g_lo + 1
                    av = ppav.tile([128, 512], FP, tag="av")
                    den = ppd.tile([128, 512], FP, tag="dn")
                    qlo = qT[0:64, pr * QSH:(pr + 1) * QSH]
                    qhi = qT[64:128, pr * QSH:(pr + 1) * QSH]
                    pend = None
                    for cc in range(NKC // 2):
                        s_lo = pps.tile([128, 1024], FP, tag="s")
                        s_hi = pps.tile([128, 1024], FP, tag="s")
                        for j in range(2):
                            c = 2 * cc + j
                            ks = slice(t * SEQ + c * 128, t * SEQ + (c + 1) * 128)
                            o = slice(j * 512, (j + 1) * 512)
                            nc.tensor.matmul(s_lo[:, o], lhsT=kT[0:64, ks], rhs=qlo)
                            nc.tensor.matmul(s_hi[:, o], lhsT=kT[64:128, ks], rhs=qhi)
                        if pend is not None:
                            _emit_av2(nc, pend, v_all, av, den, ones_c, g_lo, g_hi)
                        if cc == 6 and fin_pend is not None:
                            # previous pair's normalization, emitted deep into
                            # this pair's stream so its reciprocal is long done
                            finalize(*fin_pend)
                            fin_pend = None
                        e_lo = wexp.tile([128, 1024], BF, tag="e")
                        e_hi = wexp.tile([128, 1024], BF, tag="e")
                        nc.scalar.activation(e_lo[:], s_lo[:], AF.Exp, scale=SCALE)
                        nc.scalar.activation(e_hi[:], s_hi[:], AF.Exp, scale=SCALE)
                        pend = (cc, e_lo, e_hi)
                    _emit_av2(nc, pend, v_all, av, den, ones_c, g_lo, g_hi)
                    fin_pend = (pr, av, den)
                finalize(*fin_pend)

            # ============ phase 3: out projection ============
            with tc.tile_pool(name="ppo", bufs=2, space="PSUM") as ppo, \
                 tc.tile_pool(name="wout", bufs=3) as wout:
                for qc in range(4):
                    for nci in range(2):
                        ps = ppo.tile([128, 512], FP, tag="po")
                        for oc in range(8):
                            nc.tensor.matmul(
                                ps[:],
                                lhsT=oT[:, oc * QSH + qc * 128: oc * QSH + (qc + 1) * 128],
                                rhs=wo_s[:, oc * DIM + nci * 512: oc * DIM + (nci + 1) * 512],
                                start=(oc == 0), stop=(oc == 7 and not b_o))
                        if b_o:
                            nc.tensor.matmul(ps[:], lhsT=ones_b[:],
                                             rhs=rbo_s[:, nci * 512:(nci + 1) * 512],
                                             start=False, stop=True)
                        ot = wout.tile([128, 512], FP, tag="ot")
                        nc.any.tensor_copy(ot[:], ps[:])
                        nc.sync.dma_start(
                            out_d[qc * 128:(qc + 1) * 128, nci * 512:(nci + 1) * 512],
                            ot[:])

    nc.compile()
    return nc


def _emit_av2(nc, pend, v_all, av, den, ones_c, g_lo, g_hi):
    cc, e_lo, e_hi = pend
    for j in range(2):
        c = 2 * cc + j
        first, last = (c == 0), (c == NKC - 1)
        o = slice(j * 512, (j + 1) * 512)
        nc.tensor.matmul(av[0:64, :], lhsT=v_all[:, c * 256 + g_lo * 64: c * 256 + g_lo * 64 + 64],
                         rhs=e_lo[:, o], start=first, stop=last, skip_group_check=True)
        nc.tensor.matmul(av[64:128, :], lhsT=v_all[:, c * 256 + g_hi * 64: c * 256 + g_hi * 64 + 64],
                         rhs=e_hi[:, o], start=first, stop=last, skip_group_check=True)
        nc.tensor.matmul(den[0:1, :], lhsT=ones_c[:], rhs=e_lo[:, o],
                         start=first, stop=last, skip_group_check=True)
        nc.tensor.matmul(den[32:33, :], lhsT=ones_c[:], rhs=e_hi[:, o],
                         start=first, stop=last, skip_group_check=True)


def _get_nc(flags):
    if flags not in _cache:
        _cache[flags] = _build(*flags)
    return _cache[flags]


def _stage(query, key, value, Wq, bq, Wk, bk, Wv, bv, qnw, qnb, knw, knb, Wo, bo,
           flags):
    fq_aff, fk_aff, b_q, b_k, b_v, b_o = flags
    bf = lambda a: np.ascontiguousarray(a.astype(np.float32)).astype(BF16NP)
    wqT = bf(Wq[PERM, :].T)
    wkT = bf(Wk.T)
    wvT = bf(Wv.T)
    woT = bf(Wo[:, PERM].T)
    ident = np.eye(128, dtype=BF16NP)
    common = {"wqT": wqT, "wkT": wkT, "wvT": wvT, "woT": woT, "ident": ident}
    if fq_aff:
        common["gq"] = bf(np.tile(qnw, (128, NH)))
        common["zq"] = bf(np.tile(qnb, (128, NH)))
    if fk_aff:
        common["gk"] = bf(np.tile(knw, (128, NG)))
        common["zk"] = bf(np.tile(knb, (128, NG)))
    if b_q:
        common["rbq"] = bf(bq[PERM][None, :])
    if b_k:
        common["rbk"] = bf(bk[None, :])
    if b_v:
        common["rbv"] = bf(bv[None, :])
    if b_o:
        common["rbo"] = bf(bo[None, :])

    in_maps = []
    for core in range(NDEV):
        b, qc4 = core // 4, core % 4
        m = dict(common)
        m["xqT"] = bf(query[b, qc4 * QSH:(qc4 + 1) * QSH, :].T)
        m["xkT"] = bf(key[b].T)
        m["xvT"] = bf(value[b].T)
        in_maps.append(m)
    return in_maps


def _flags(bq, bk, bv, bo, qnw, qnb, knw, knb):
    return (not (np.all(qnw == 1.0) and np.all(qnb == 0.0)),
            not (np.all(knw == 1.0) and np.all(knb == 0.0)),
            bool(np.any(bq != 0.0)), bool(np.any(bk != 0.0)),
            bool(np.any(bv != 0.0)), bool(np.any(bo != 0.0)))


def _numpy_ref(query, key, value, attn_mask, Wq, bq, Wk, bk, Wv, bv,
               qnw, qnb, knw, knb, Wo, bo):
    def ln(x, w, b):
        m = x.mean(-1, keepdims=True)
        v = np.square(x - m).mean(-1, keepdims=True)
        return (x - m) / np.sqrt(v + LN_EPS) * w + b

    Bn, Q, _ = query.shape
    KV = key.shape[1]
    q = query @ Wq.T + bq
    k = key @ Wk.T + bk
    v = value @ Wv.T + bv
    q = q.reshape(Bn, Q, NG, HPG, HD).transpose(0, 2, 3, 1, 4)
    k = k.reshape(Bn, KV, NG, HD).transpose(0, 2, 1, 3)
    v = v.reshape(Bn, KV, NG, HD).transpose(0, 2, 1, 3)
    q = ln(q, qnw, qnb)
    k = ln(k, knw, knb)
    out = np.zeros((Bn, NG, HPG, Q, HD), np.float32)
    for b in range(Bn):
        for g in range(NG):
            for h in range(HPG):
                s = (q[b, g, h] @ k[b, g].T) * SCALE
                s = np.where(attn_mask[b], s, np.float32(np.finfo(np.float32).min))
                s -= s.max(-1, keepdims=True)
                e = np.exp(s)
                a = e / e.sum(-1, keepdims=True)
                out[b, g, h] = a @ v[b, g]
    out = out.transpose(0, 3, 1, 2, 4).reshape(Bn, Q, DIM)
    return (out @ Wo.T + bo).astype(np.float32)


def kernel(query, key, value, attn_mask, Wq, bq, Wk, bk, Wv, bv,
           q_norm_w, q_norm_b, k_norm_w, k_norm_b, Wo, bo):
    f32 = lambda a: np.asarray(a, np.float32)
    query, key, value = f32(query), f32(key), f32(value)
    Wq, bq, Wk, bk = f32(Wq), f32(bq), f32(Wk), f32(bk)
    Wv, bv, Wo, bo = f32(Wv), f32(bv), f32(Wo), f32(bo)
    qnw, qnb, knw, knb = f32(q_norm_w), f32(q_norm_b), f32(k_norm_w), f32(k_norm_b)
    attn_mask = np.asarray(attn_mask, bool)

    if not attn_mask.all():
        return _numpy_ref(query, key, value, attn_mask, Wq, bq, Wk, bk, Wv, bv,
                          qnw, qnb, knw, knb, Wo, bo)

    flags = _flags(bq, bk, bv, bo, qnw, qnb, knw, knb)
    nc = _get_nc(flags)
    in_maps = _stage(query, key, value, Wq, bq, Wk, bk, Wv, bv,
                     qnw, qnb, knw, knb, Wo, bo, flags)
    from concourse.bass_utils import run_bass_kernel_spmd
    res = run_bass_kernel_spmd(nc, in_maps, core_ids=list(range(NDEV)))
    out = np.empty((B, SEQ, DIM), np.float32)
    for core in range(NDEV):
        b, qc4 = core // 4, core % 4
        out[b, qc4 * QSH:(qc4 + 1) * QSH, :] = res.results[core]["out"]
    return out


def run_traced(inputs, tmpdir=None):
    """test.py helper: run once with NTFF tracing, return (out, results)."""
    f32 = lambda a: np.asarray(a, np.float32)
    flags = _flags(f32(inputs["bq"]), f32(inputs["bk"]), f32(inputs["bv"]),
                   f32(inputs["bo"]), f32(inputs["q_norm_w"]), f32(inputs["q_norm_b"]),
                   f32(inputs["k_norm_w"]), f32(inputs["k_norm_b"]))
    nc = _get_nc(flags)
    in_maps = _stage(f32(inputs["query"]), f32(inputs["key"]), f32(inputs["value"]),
                     f32(inputs["Wq"]), f32(inputs["bq"]), f32(inputs["Wk"]),
                     f32(inputs["bk"]), f32(inputs["Wv"]), f32(inputs["bv"]),
                     f32(inputs["q_norm_w"]), f32(inputs["q_norm_b"]),
                     f32(inputs["k_norm_w"]), f32(inputs["k_norm_b"]),
                     f32(inputs["Wo"]), f32(inputs["bo"]), flags)
    from concourse.bass_utils import run_bass_kernel_spmd
    res = run_bass_kernel_spmd(nc, in_maps, core_ids=list(range(NDEV)),
                               trace=True, tmpdir=tmpdir)
    out = np.empty((B, SEQ, DIM), np.float32)
    for core in range(NDEV):
        b, qc4 = core // 4, core % 4
        out[b, qc4 * QSH:(qc4 + 1) * QSH, :] = res.results[core]["out"]
    return out, res


# revision 46
# speedup vs baseline: 1.2175x; 1.2175x over previous
"""GroupedQueryAttention Bass/Tile kernel for 8 TRN2 NeuronCores.

Sharding: data-parallel over (batch, query-quarter): core i -> batch i//4,
query rows (i%4)*512 .. +512. Each core computes all 16 heads for its 512
query rows fully on-device: QKV projections, QK-LayerNorm, attention in a
transposed score layout (scoresT[kv, q], so attn@V needs no transposes),
softmax denominators via ones-column matmuls col-tiled next to the AV
matmuls, then the out-projection. The 16 heads are processed as 8
row-tiled pairs on the 128x128 PE array (head_dim=64 -> two heads share
the contraction rows). Host work is layout staging only (transpose / bf16
cast / head permutation) plus the final shard concat.
"""
import sys

if "/opt/trn_rl_repo" not in sys.path:
    sys.path.insert(0, "/opt/trn_rl_repo")

import numpy as np
import ml_dtypes

BF16NP = ml_dtypes.bfloat16
DIM = 1024
NH = 16
NG = 4
HD = 64
HPG = 4
B = 2
SEQ = 2048
QSH = 512          # query rows per core
NDEV = 8
NPAIR = 8          # head pairs
NKC = 16           # kv chunks of 128
LN_EPS = 1e-5
SCALE = 1.0 / 8.0  # 1/sqrt(HD)


def _make_perm():
    # head pair p holds orig heads (g_lo, h) and (g_hi, h) in its low/high
    # 64 partitions, where g_lo = 2*(p//4), g_hi = g_lo+1, h = p%4.
    p_ = np.zeros(DIM, np.int64)
    for pr in range(NPAIR):
        for half in range(2):
            g = 2 * (pr // 4) + half
            h = pr % 4
            src = g * (HPG * HD) + h * HD
            dst = pr * 128 + half * 64
            p_[dst:dst + 64] = np.arange(src, src + 64)
    return p_


PERM = _make_perm()

_cache = {}


def _build(fq_aff, fk_aff, b_q, b_k, b_v, b_o):
    """Build + compile the single-core Bass graph (shared by all 8 cores)."""
    import concourse.bass as bass
    import concourse.tile as tile
    from concourse import bacc, mybir

    dt = mybir.dt
    FP, BF = dt.float32, dt.bfloat16
    AF = mybir.ActivationFunctionType
    OP = mybir.AluOpType

    nc = bacc.Bacc("TRN2", target_bir_lowering=False, debug=False)

    def din(name, shape, dtype=BF):
        return nc.dram_tensor(name, shape, dtype, kind="ExternalInput").ap()

    xq = din("xqT", [DIM, QSH])
    xk = din("xkT", [DIM, SEQ])
    xv = din("xvT", [DIM, SEQ])
    wq = din("wqT", [DIM, DIM])
    wk = din("wkT", [DIM, 256])
    wv = din("wvT", [DIM, 256])
    wo = din("woT", [DIM, DIM])
    idn = din("ident", [128, 128])
    gq = din("gq", [128, DIM]) if fq_aff else None
    zq = din("zq", [128, DIM]) if fq_aff else None
    gk = din("gk", [128, 256]) if fk_aff else None
    zk = din("zk", [128, 256]) if fk_aff else None
    rbq = din("rbq", [1, DIM]) if b_q else None
    rbk = din("rbk", [1, 256]) if b_k else None
    rbv = din("rbv", [1, 256]) if b_v else None
    rbo = din("rbo", [1, DIM]) if b_o else None
    out_d = nc.dram_tensor("out", [QSH, DIM], FP, kind="ExternalOutput").ap()

    with tile.TileContext(nc) as tc:
        from contextlib import ExitStack
        with ExitStack() as ctx:
            pers = ctx.enter_context(tc.tile_pool(name="pers", bufs=1))
            wexp = ctx.enter_context(tc.tile_pool(name="wexp", bufs=4))
            wrec = ctx.enter_context(tc.tile_pool(name="wrec", bufs=2))

            # ---- stage inputs into SBUF ----
            xq_s = pers.tile([128, 8 * QSH], BF, tag="xq")
            xk_s = pers.tile([128, 8 * SEQ], BF, tag="xk")
            xv_s = pers.tile([128, 8 * SEQ], BF, tag="xv")
            wq_s = pers.tile([128, 8 * DIM], BF, tag="wq")
            wk_s = pers.tile([128, 8 * 256], BF, tag="wk")
            wv_s = pers.tile([128, 8 * 256], BF, tag="wv")
            wo_s = pers.tile([128, 8 * DIM], BF, tag="wo")
            id_s = pers.tile([128, 128], BF, tag="ident")
            nc.sync.dma_start(id_s[:], idn[:])
            # q-projection inputs first so the PE can start ~10us in, with
            # the k/v/wo streams landing under the early compute.
            for mc in range(8):
                r = slice(mc * 128, (mc + 1) * 128)
                nc.sync.dma_start(wq_s[:, mc * DIM:(mc + 1) * DIM], wq[r, :])
                nc.sync.dma_start(xq_s[:, mc * QSH:(mc + 1) * QSH], xq[r, :])
            for mc in range(8):
                r = slice(mc * 128, (mc + 1) * 128)
                nc.sync.dma_start(wk_s[:, mc * 256:(mc + 1) * 256], wk[r, :])
                nc.sync.dma_start(wv_s[:, mc * 256:(mc + 1) * 256], wv[r, :])
                nc.sync.dma_start(xk_s[:, mc * SEQ:(mc + 1) * SEQ], xk[r, :])
            for mc in range(8):
                r = slice(mc * 128, (mc + 1) * 128)
                nc.sync.dma_start(xv_s[:, mc * SEQ:(mc + 1) * SEQ], xv[r, :])
            for mc in range(8):
                r = slice(mc * 128, (mc + 1) * 128)
                nc.sync.dma_start(wo_s[:, mc * DIM:(mc + 1) * DIM], wo[r, :])
            gq_s = zq_s = gk_s = zk_s = None
            if fq_aff:
                gq_s = pers.tile([128, DIM], BF, tag="gq")
                zq_s = pers.tile([128, DIM], BF, tag="zq")
                nc.sync.dma_start(gq_s[:], gq[:])
                nc.sync.dma_start(zq_s[:], zq[:])
            if fk_aff:
                gk_s = pers.tile([128, 256], BF, tag="gk")
                zk_s = pers.tile([128, 256], BF, tag="zk")
                nc.sync.dma_start(gk_s[:], gk[:])
                nc.sync.dma_start(zk_s[:], zk[:])
            rbq_s = rbk_s = rbv_s = rbo_s = ones_b = None
            if b_q or b_k or b_v or b_o:
                ones_b = pers.tile([1, 128], BF, tag="ones_b")
                nc.vector.memset(ones_b[:], 1.0)
            if b_q:
                rbq_s = pers.tile([1, DIM], BF, tag="rbq")
                nc.sync.dma_start(rbq_s[:], rbq[:])
            if b_k:
                rbk_s = pers.tile([1, 256], BF, tag="rbk")
                nc.sync.dma_start(rbk_s[:], rbk[:])
            if b_v:
                rbv_s = pers.tile([1, 256], BF, tag="rbv")
                nc.sync.dma_start(rbv_s[:], rbv[:])
            if b_o:
                rbo_s = pers.tile([1, DIM], BF, tag="rbo")
                nc.sync.dma_start(rbo_s[:], rbo[:])

            ones_c = pers.tile([128, 1], BF, tag="ones_c")    # den lhsT
            nc.vector.memset(ones_c[:], 1.0)
            ones_r = pers.tile([128, 64], BF, tag="ones_r")   # bcast lhsT
            nc.vector.memset(ones_r[:], 1.0)

            q_nat = pers.tile([128, 4 * DIM], BF, tag="q_nat")
            k_nat = pers.tile([128, 16 * 256], BF, tag="k_nat")
            v_all = pers.tile([128, 16 * 256], BF, tag="v_all")
            qT = pers.tile([128, NPAIR * QSH], BF, tag="qT")
            kT = pers.tile([128, 2 * SEQ], BF, tag="kT")
            oT = pers.tile([128, NPAIR * QSH], BF, tag="oT")

            # ============ phase 1: projections + LN + transposes ============
            with tc.tile_pool(name="pp1", bufs=2, space="PSUM") as pp1, \
                 tc.tile_pool(name="ppt", bufs=4, space="PSUM") as ppt:

                # q projection -> q_nat[q(128 x 4qc), qc*1024 + o']
                # nci inner so consecutive matmuls share the same stationary
                # lhsT slice of xq
                for qc in range(4):
                    pss = [pp1.tile([128, 512], FP, tag="pj", name=f"psq_{qc}_{i}") for i in range(2)]
                    for mc in range(8):
                        for nci in range(2):
                            nc.tensor.matmul(
                                pss[nci][:],
                                lhsT=xq_s[:, mc * QSH + qc * 128: mc * QSH + (qc + 1) * 128],
                                rhs=wq_s[:, mc * DIM + nci * 512: mc * DIM + (nci + 1) * 512],
                                start=(mc == 0), stop=(mc == 7 and not b_q))
                    for nci in range(2):
                        if b_q:
                            nc.tensor.matmul(pss[nci][:], lhsT=ones_b[:],
                                             rhs=rbq_s[:, nci * 512:(nci + 1) * 512],
                                             start=False, stop=True)
                        nc.scalar.copy(q_nat[:, qc * DIM + nci * 512: qc * DIM + (nci + 1) * 512], pss[nci][:])

                # k projection -> k_nat[kv(128 x 16kc), kc*256 + gd]
                # v projection -> v_all (same layout)
                for (x_s, w_s, rb_s, dst, nz) in ((xk_s, wk_s, rbk_s, k_nat, b_k),
                                                  (xv_s, wv_s, rbv_s, v_all, b_v)):
                    for ic in range(8):
                        ps = pp1.tile([128, 512], FP, tag="pj")
                        for half in range(2):
                            c = 2 * ic + half
                            o = slice(half * 256, (half + 1) * 256)
                            for mc in range(8):
                                nc.tensor.matmul(
                                    ps[:, o],
                                    lhsT=x_s[:, mc * SEQ + c * 128: mc * SEQ + (c + 1) * 128],
                                    rhs=w_s[:, mc * 256:(mc + 1) * 256],
                                    start=(mc == 0), stop=(mc == 7 and not nz))
                            if nz:
                                nc.tensor.matmul(ps[:, o], lhsT=ones_b[:], rhs=rb_s[:],
                                                 start=False, stop=True)
                        nc.scalar.copy(dst[:, ic * 512:(ic + 1) * 512], ps[:])

                # ---- LayerNorm over head_dim (free-dim segments of 64) ----
                # Stats via DVE reduces; the per-segment (scale, shift) pair is
                # broadcast to full [128, 4096] tiles on the otherwise-idle
                # GPSIMD engine (step-0 inner AP), then applied with two big
                # 2x-mode tensor_tensor ops. scrA/scrB reuse dead tiles.
                NSEG = 64

                def bcast64(ap):
                    flat = ap.rearrange("p a b -> p (a b)")
                    return bass.AP(flat.tensor, flat.offset, flat.ap.copy() + [[0, HD]])

                sqb = pers.tile([128, 4096], BF, tag="sqb")
                for (big, tg, g_s, z_s, aff, ngrp) in (
                        (q_nat, "q", gq_s, zq_s, fq_aff, 4),
                        (k_nat, "k", gk_s, zk_s, fk_aff, 16)):
                    mean = pers.tile([128, NSEG, 1], FP, tag="mn_" + tg)
                    veps = pers.tile([128, NSEG, 1], FP, tag="ve_" + tg)
                    aa = pers.tile([128, NSEG, 1], FP, tag="aa_" + tg)
                    bb = pers.tile([128, NSEG, 1], FP, tag="bb_" + tg)
                    nc.scalar.activation(sqb[:], big[:], AF.Square)
                    nc.vector.reduce_sum(
                        mean[:], big[:].rearrange("p (g d) -> p g d", d=HD),
                        axis=mybir.AxisListType.X)
                    nc.vector.reduce_sum(
                        veps[:], sqb[:].rearrange("p (g d) -> p g d", d=HD),
                        axis=mybir.AxisListType.X)
                    nc.vector.tensor_scalar(mean[:], mean[:], 1.0 / HD, None, OP.mult)
                    nc.vector.tensor_scalar(veps[:], veps[:], 1.0 / HD, LN_EPS,
                                            OP.mult, OP.add)
                    nc.vector.tensor_tensor(bb[:], mean[:], mean[:], op=OP.mult)
                    nc.vector.tensor_tensor(veps[:], veps[:], bb[:], op=OP.subtract)
                    nc.scalar.activation(veps[:], veps[:], AF.Sqrt)
                    nc.vector.reciprocal(aa[:], veps[:])
                    nc.vector.tensor_tensor(bb[:], mean[:], aa[:], op=OP.mult)
                    nc.vector.tensor_scalar(bb[:], bb[:], -1.0, None, OP.mult)
                    # apply per 1024-col chunk so the PE transposes of early
                    # chunks can start while later chunks are still applying
                    for ch in range(4):
                        sl = slice(ch * 1024, (ch + 1) * 1024)
                        big3 = big[:, sl].rearrange("p (g d) -> p g d", d=HD)
                        sla = slice(ch * 16, (ch + 1) * 16)
                        nc.vector.tensor_tensor(big3, big3, bcast64(aa[:, sla, :]), op=OP.mult)
                        nc.vector.tensor_tensor(big3, big3, bcast64(bb[:, sla, :]), op=OP.add)
                    if aff:
                        w = 4096 // ngrp
                        for s in range(ngrp):
                            sl = slice(s * w, (s + 1) * w)
                            nc.vector.tensor_tensor(big[:, sl], big[:, sl], g_s[:, 0:w], op=OP.mult)
                            nc.vector.tensor_tensor(big[:, sl], big[:, sl], z_s[:, 0:w], op=OP.add)

                # ---- PE transposes: q_nat -> qT, k_nat -> kT ----
                for qc in range(4):
                    for pr in range(8):
                        pt = ppt.tile([128, 128], BF, tag="tr")
                        nc.tensor.transpose(pt[:], q_nat[:, qc * DIM + pr * 128: qc * DIM + (pr + 1) * 128], id_s[:])
                        nc.any.tensor_copy(qT[:, pr * QSH + qc * 128: pr * QSH + (qc + 1) * 128], pt[:])
                for t in range(2):
                    for kc in range(16):
                        pt = ppt.tile([128, 128], BF, tag="tr")
                        nc.tensor.transpose(pt[:], k_nat[:, kc * 256 + t * 128: kc * 256 + (t + 1) * 128], id_s[:])
                        nc.any.tensor_copy(kT[:, t * SEQ + kc * 128: t * SEQ + (kc + 1) * 128], pt[:])

            # ============ phase 2: attention ============
            with tc.tile_pool(name="pps", bufs=2, space="PSUM") as pps, \
                 tc.tile_pool(name="ppav", bufs=2, space="PSUM") as ppav, \
                 tc.tile_pool(name="ppd", bufs=2, space="PSUM") as ppd:
                def finalize(pr, av, den):
                    # bf16-cast the two denominator rows, broadcast them over
                    # the pair's 64-partition halves with cheap bf16 matmuls,
                    # then one full-width reciprocal (cost is FD-bound) + mult.
                    dcb = wrec.tile([128, 512], BF, tag="dcb")
                    nc.vector.tensor_copy(dcb[0:1, :], den[0:1, :])
                    nc.vector.tensor_copy(dcb[32:33, :], den[32:33, :])
                    bc = ppd.tile([128, 512], FP, tag="dn")
                    nc.tensor.matmul(bc[0:64, :], lhsT=ones_r[0:1, :], rhs=dcb[0:1, :])
                    nc.tensor.matmul(bc[64:128, :], lhsT=ones_r[32:33, :], rhs=dcb[32:33, :])
                    rcb = wrec.tile([128, 512], FP, tag="rc")
                    nc.vector.reciprocal(rcb[:], bc[:])
                    nc.vector.tensor_tensor(oT[:, pr * QSH:(pr + 1) * QSH], av[:], rcb[:], op=OP.mult)

                fin_pend = None
                for pr in range(NPAIR):
                    t = pr // 4
                    g_lo = 2 * (pr // 4)
                    g_hi = g_lo + 1
                    av = ppav.tile([128, 512], FP, tag="av")
                    den = ppd.tile([128, 512], FP, tag="dn")
                    qlo = qT[0:64, pr * QSH:(pr + 1) * QSH]
                    qhi = qT[64:128, pr * QSH:(pr + 1) * QSH]
                    pend = None
                    for cc in range(NKC // 2):
                        s_lo = pps.tile([128, 1024], FP, tag="s")
                        s_hi = pps.tile([128, 1024], FP, tag="s")
                        for half in range(2):
                            for j in range(2):
                                c = 2 * cc + j
                                ks = slice(t * SEQ + c * 128, t * SEQ + (c + 1) * 128)
                                o = slice(j * 512, (j + 1) * 512)
                                if half == 0:
                                    nc.tensor.matmul(s_lo[:, o], lhsT=kT[0:64, ks], rhs=qlo)
                                else:
                                    nc.tensor.matmul(s_hi[:, o], lhsT=kT[64:128, ks], rhs=qhi)
                        if pend is not None:
                            _emit_av2(nc, pend, v_all, av, den, ones_c, g_lo, g_hi)
                        if cc == 6 and fin_pend is not None:
                            # previous pair's normalization, emitted deep into
                            # this pair's stream so its reciprocal is long done
                            finalize(*fin_pend)
                            fin_pend = None
                        e_lo = wexp.tile([128, 1024], BF, tag="e")
                        e_hi = wexp.tile([128, 1024], BF, tag="e")
                        nc.scalar.activation(e_lo[:], s_lo[:], AF.Exp, scale=SCALE)
                        nc.scalar.activation(e_hi[:], s_hi[:], AF.Exp, scale=SCALE)
                        pend = (cc, e_lo, e_hi)
                    _emit_av2(nc, pend, v_all, av, den, ones_c, g_lo, g_hi)
                    fin_pend = (pr, av, den)
                finalize(*fin_pend)

            # ============ phase 3: out projection ============
            with tc.tile_pool(name="ppo", bufs=2, space="PSUM") as ppo, \
                 tc.tile_pool(name="wout", bufs=3) as wout:
                for qc in range(4):
                    for nci in range(2):
                        ps = ppo.tile([128, 512], FP, tag="po")
                        for oc in range(8):
                            nc.tensor.matmul(
                                ps[:],
                                lhsT=oT[:, oc * QSH + qc * 128: oc * QSH + (qc + 1) * 128],
                                rhs=wo_s[:, oc * DIM + nci * 512: oc * DIM + (nci + 1) * 512],
                                start=(oc == 0), stop=(oc == 7 and not b_o))
                        if b_o:
                            nc.tensor.matmul(ps[:], lhsT=ones_b[:],
                                             rhs=rbo_s[:, nci * 512:(nci + 1) * 512],
                                             start=False, stop=True)
                        ot = wout.tile([128, 512], FP, tag="ot")
                        nc.any.tensor_copy(ot[:], ps[:])
                        nc.sync.dma_start(
                            out_d[qc * 128:(qc + 1) * 128, nci * 512:(nci + 1) * 512],
                            ot[:])

    nc.compile()
    return nc


def _emit_av2(nc, pend, v_all, av, den, ones_c, g_lo, g_hi):
    cc, e_lo, e_hi = pend
    for j in range(2):
        c = 2 * cc + j
        first, last = (c == 0), (c == NKC - 1)
        o = slice(j * 512, (j + 1) * 512)
        nc.tensor.matmul(av[0:64, :], lhsT=v_all[:, c * 256 + g_lo * 64: c * 256 + g_lo * 64 + 64],
                         rhs=e_lo[:, o], start=first, stop=last, skip_group_check=True)
        nc.tensor.matmul(av[64:128, :], lhsT=v_all[:, c * 256 + g_hi * 64: c * 256 + g_hi * 64 + 64],
                         rhs=e_hi[:, o], start=first, stop=last, skip_group_check=True)
        nc.tensor.matmul(den[0:1, :], lhsT=ones_c[:], rhs=e_lo[:, o],
                         start=first, stop=last, skip_group_check=True)
        nc.tensor.matmul(den[32:33, :], lhsT=ones_c[:], rhs=e_hi[:, o],
                         start=first, stop=last, skip_group_check=True)


def _get_nc(flags):
    if flags not in _cache:
        _cache[flags] = _build(*flags)
    return _cache[flags]


def _stage(query, key, value, Wq, bq, Wk, bk, Wv, bv, qnw, qnb, knw, knb, Wo, bo,
           flags):
    fq_aff, fk_aff, b_q, b_k, b_v, b_o = flags
    bf = lambda a: np.ascontiguousarray(a.astype(np.float32)).astype(BF16NP)
    wqT = bf(Wq[PERM, :].T)
    wkT = bf(Wk.T)
    wvT = bf(Wv.T)
    woT = bf(Wo[:, PERM].T)
    ident = np.eye(128, dtype=BF16NP)
    common = {"wqT": wqT, "wkT": wkT, "wvT": wvT, "woT": woT, "ident": ident}
    if fq_aff:
        common["gq"] = bf(np.tile(qnw, (128, NH)))
        common["zq"] = bf(np.tile(qnb, (128, NH)))
    if fk_aff:
        common["gk"] = bf(np.tile(knw, (128, NG)))
        common["zk"] = bf(np.tile(knb, (128, NG)))
    if b_q:
        common["rbq"] = bf(bq[PERM][None, :])
    if b_k:
        common["rbk"] = bf(bk[None, :])
    if b_v:
        common["rbv"] = bf(bv[None, :])
    if b_o:
        common["rbo"] = bf(bo[None, :])

    in_maps = []
    for core in range(NDEV):
        b, qc4 = core // 4, core % 4
        m = dict(common)
        m["xqT"] = bf(query[b, qc4 * QSH:(qc4 + 1) * QSH, :].T)
        m["xkT"] = bf(key[b].T)
        m["xvT"] = bf(value[b].T)
        in_maps.append(m)
    return in_maps


def _flags(bq, bk, bv, bo, qnw, qnb, knw, knb):
    return (not (np.all(qnw == 1.0) and np.all(qnb == 0.0)),
            not (np.all(knw == 1.0) and np.all(knb == 0.0)),
            bool(np.any(bq != 0.0)), bool(np.any(bk != 0.0)),
            bool(np.any(bv != 0.0)), bool(np.any(bo != 0.0)))


def _numpy_ref(query, key, value, attn_mask, Wq, bq, Wk, bk, Wv, bv,
               qnw, qnb, knw, knb, Wo, bo):
    def ln(x, w, b):
        m = x.mean(-1, keepdims=True)
        v = np.square(x - m).mean(-1, keepdims=True)
        return (x - m) / np.sqrt(v + LN_EPS) * w + b

    Bn, Q, _ = query.shape
    KV = key.shape[1]
    q = query @ Wq.T + bq
    k = key @ Wk.T + bk
    v = value @ Wv.T + bv
    q = q.reshape(Bn, Q, NG, HPG, HD).transpose(0, 2, 3, 1, 4)
    k = k.reshape(Bn, KV, NG, HD).transpose(0, 2, 1, 3)
    v = v.reshape(Bn, KV, NG, HD).transpose(0, 2, 1, 3)
    q = ln(q, qnw, qnb)
    k = ln(k, knw, knb)
    out = np.zeros((Bn, NG, HPG, Q, HD), np.float32)
    for b in range(Bn):
        for g in range(NG):
            for h in range(HPG):
                s = (q[b, g, h] @ k[b, g].T) * SCALE
                s = np.where(attn_mask[b], s, np.float32(np.finfo(np.float32).min))
                s -= s.max(-1, keepdims=True)
                e = np.exp(s)
                a = e / e.sum(-1, keepdims=True)
                out[b, g, h] = a @ v[b, g]
    out = out.transpose(0, 3, 1, 2, 4).reshape(Bn, Q, DIM)
    return (out @ Wo.T + bo).astype(np.float32)


def kernel(query, key, value, attn_mask, Wq, bq, Wk, bk, Wv, bv,
           q_norm_w, q_norm_b, k_norm_w, k_norm_b, Wo, bo):
    f32 = lambda a: np.asarray(a, np.float32)
    query, key, value = f32(query), f32(key), f32(value)
    Wq, bq, Wk, bk = f32(Wq), f32(bq), f32(Wk), f32(bk)
    Wv, bv, Wo, bo = f32(Wv), f32(bv), f32(Wo), f32(bo)
    qnw, qnb, knw, knb = f32(q_norm_w), f32(q_norm_b), f32(k_norm_w), f32(k_norm_b)
    attn_mask = np.asarray(attn_mask, bool)

    if not attn_mask.all():
        return _numpy_ref(query, key, value, attn_mask, Wq, bq, Wk, bk, Wv, bv,
                          qnw, qnb, knw, knb, Wo, bo)

    flags = _flags(bq, bk, bv, bo, qnw, qnb, knw, knb)
    nc = _get_nc(flags)
    in_maps = _stage(query, key, value, Wq, bq, Wk, bk, Wv, bv,
                     qnw, qnb, knw, knb, Wo, bo, flags)
    from concourse.bass_utils import run_bass_kernel_spmd
    res = run_bass_kernel_spmd(nc, in_maps, core_ids=list(range(NDEV)))
    out = np.empty((B, SEQ, DIM), np.float32)
    for core in range(NDEV):
        b, qc4 = core // 4, core % 4
        out[b, qc4 * QSH:(qc4 + 1) * QSH, :] = res.results[core]["out"]
    return out


def run_traced(inputs, tmpdir=None):
    """test.py helper: run once with NTFF tracing, return (out, results)."""
    f32 = lambda a: np.asarray(a, np.float32)
    flags = _flags(f32(inputs["bq"]), f32(inputs["bk"]), f32(inputs["bv"]),
                   f32(inputs["bo"]), f32(inputs["q_norm_w"]), f32(inputs["q_norm_b"]),
                   f32(inputs["k_norm_w"]), f32(inputs["k_norm_b"]))
    nc = _get_nc(flags)
    in_maps = _stage(f32(inputs["query"]), f32(inputs["key"]), f32(inputs["value"]),
                     f32(inputs["Wq"]), f32(inputs["bq"]), f32(inputs["Wk"]),
                     f32(inputs["bk"]), f32(inputs["Wv"]), f32(inputs["bv"]),
                     f32(inputs["q_norm_w"]), f32(inputs["q_norm_b"]),
                     f32(inputs["k_norm_w"]), f32(inputs["k_norm_b"]),
                     f32(inputs["Wo"]), f32(inputs["bo"]), flags)
    from concourse.bass_utils import run_bass_kernel_spmd
    res = run_bass_kernel_spmd(nc, in_maps, core_ids=list(range(NDEV)),
                               trace=True, tmpdir=tmpdir)
    out = np.empty((B, SEQ, DIM), np.float32)
    for core in range(NDEV):
        b, qc4 = core // 4, core % 4
        out[b, qc4 * QSH:(qc4 + 1) * QSH, :] = res.results[core]["out"]
    return out, res
